# revision 1
# baseline (speedup 1.0000x reference)
"""MEGNet (3 GN blocks + Set2Set + head) with the dominant dense compute
(edge/node two-layer MLPs) running as a Bass/Tile SPMD kernel on 8 trn2
NeuronCores. Rows are sharded evenly across cores; host does the index
gathers/segment reductions and the tiny (B=64) global/Set2Set/head math.
"""

import numpy as np
import ml_dtypes

N_BLOCKS = 3
N_CORES = 8
P = 128
F = 512  # free-dim tile (rows per matmul)

_CACHE = {}


def _build_mlp_nc(M, K, H, O):
    """Two-layer linear kernel: outT = W2 @ (W1 @ xT + b1) + b2.

    xT: (K, M) bf16 input (feature-major), out: (O, M) f32.
    No activation (reference _linear chains have none).
    """
    import concourse.bacc as bacc
    import concourse.mybir as mybir
    from concourse.tile import TileContext

    nc = bacc.Bacc()
    dt = mybir.dt
    xT = nc.dram_tensor("xT", [K, M], dt.bfloat16, kind="ExternalInput")
    w1 = nc.dram_tensor("w1", [K, H], dt.bfloat16, kind="ExternalInput")
    b1 = nc.dram_tensor("b1", [H, 1], dt.float32, kind="ExternalInput")
    w2 = nc.dram_tensor("w2", [H, O], dt.bfloat16, kind="ExternalInput")
    b2 = nc.dram_tensor("b2", [O, 1], dt.float32, kind="ExternalInput")
    outT = nc.dram_tensor("outT", [O, M], dt.float32, kind="ExternalOutput")

    # contraction chunks over K (partition dim <= 128)
    kchunks = []
    k0 = 0
    while k0 < K:
        kc = min(128, K - k0)
        kchunks.append((k0, kc))
        k0 += kc

    with TileContext(nc) as tc:
        with (
            tc.tile_pool(name="const", bufs=1) as cpool,
            tc.tile_pool(name="xin", bufs=4) as xin,
            tc.tile_pool(name="hid", bufs=3) as hid,
            tc.tile_pool(name="out", bufs=3) as opool,
            tc.tile_pool(name="ps1", bufs=2, space="PSUM") as ps1,
            tc.tile_pool(name="ps2", bufs=2, space="PSUM") as ps2,
        ):
            # stationary weights, biases
            w1_t = cpool.tile([128, len(kchunks) * H], dt.bfloat16, tag="w1")
            for i, (k0, kc) in enumerate(kchunks):
                nc.sync.dma_start(
                    out=w1_t[:kc, i * H : i * H + H], in_=w1[k0 : k0 + kc, :]
                )
            w2_t = cpool.tile([H, O], dt.bfloat16, tag="w2")
            nc.sync.dma_start(out=w2_t[:], in_=w2[:])
            b1_t = cpool.tile([H, 1], dt.float32, tag="b1")
            nc.sync.dma_start(out=b1_t[:], in_=b1[:])
            b2_t = cpool.tile([O, 1], dt.float32, tag="b2")
            nc.sync.dma_start(out=b2_t[:], in_=b2[:])

            # group row-tiles so each input DMA moves GRP*F columns at once
            n_tiles = M // F
            GRP = 2 if n_tiles % 2 == 0 else 1
            fused = K % 128 == 0 and len(kchunks) > 1
            xTr = (
                xT.rearrange("(a p) m -> p a m", a=len(kchunks)) if fused else None
            )
            for tg in range(n_tiles // GRP):
                g0 = tg * GRP * F
                if fused:
                    # one ~1MB DMA for all K-chunks of this group
                    x_all = xin.tile(
                        [128, len(kchunks), GRP * F], dt.bfloat16, tag="x"
                    )
                    nc.gpsimd.dma_start(
                        out=x_all[:], in_=xTr[:, :, g0 : g0 + GRP * F]
                    )
                    x_grp = [x_all[:, i, :] for i in range(len(kchunks))]
                else:
                    x_grp = []
                    for i, (k0, kc) in enumerate(kchunks):
                        x_t = xin.tile([128, GRP * F], dt.bfloat16, tag="x")
                        nc.gpsimd.dma_start(
                            out=x_t[:kc, :], in_=xT[k0 : k0 + kc, g0 : g0 + GRP * F]
                        )
                        x_grp.append(x_t)
                for j in range(GRP):
                    sl = slice(g0 + j * F, g0 + (j + 1) * F)
                    h_ps = ps1.tile([H, F], dt.float32, space="PSUM")
                    for i, (k0, kc) in enumerate(kchunks):
                        nc.tensor.matmul(
                            out=h_ps[:],
                            lhsT=w1_t[:kc, i * H : i * H + H],
                            rhs=x_grp[i][:kc, j * F : (j + 1) * F],
                            start=(i == 0),
                            stop=(i == len(kchunks) - 1),
                        )
                    h_sb = hid.tile([H, F], dt.bfloat16, tag="h")
                    nc.vector.tensor_add(
                        out=h_sb[:], in0=h_ps[:], in1=b1_t[:].to_broadcast([H, F])
                    )
                    o_ps = ps2.tile([O, F], dt.float32, space="PSUM")
                    nc.tensor.matmul(
                        out=o_ps[:], lhsT=w2_t[:], rhs=h_sb[:], start=True, stop=True
                    )
                    o_sb = opool.tile([O, F], dt.float32, tag="o")
                    nc.vector.tensor_add(
                        out=o_sb[:], in0=o_ps[:], in1=b2_t[:].to_broadcast([O, F])
                    )
                    nc.gpsimd.dma_start(out=outT[:, sl], in_=o_sb[:])
    nc.finalize()
    return nc


def _run_mlp(xT_full, W1, bias1, W2, bias2):
    """xT_full: (K, Mtot) f32. Returns (O, Mtot) f32 = W2@(W1@x+b1)+b2 via 8-core SPMD."""
    from concourse.bass_utils import run_bass_kernel_spmd

    K, Mtot = xT_full.shape
    H, O = W1.shape[0], W2.shape[0]
    Mper = -(-Mtot // (N_CORES * F)) * F  # per-core rows, padded to F
    key = (Mper, K, H, O)
    if key not in _CACHE:
        _CACHE[key] = _build_mlp_nc(Mper, K, H, O)
    nc = _CACHE[key]

    bf = ml_dtypes.bfloat16
    w1 = np.ascontiguousarray(W1.T.astype(bf))  # (K,H)
    w2 = np.ascontiguousarray(W2.T.astype(bf))  # (H,O)
    b1c = np.ascontiguousarray(bias1.astype(np.float32).reshape(H, 1))
    b2c = np.ascontiguousarray(bias2.astype(np.float32).reshape(O, 1))
    pad = Mper * N_CORES - Mtot
    xp = np.pad(xT_full, ((0, 0), (0, pad))).astype(bf)
    in_maps = []
    for c in range(N_CORES):
        in_maps.append(
            {
                "xT": np.ascontiguousarray(xp[:, c * Mper : (c + 1) * Mper]),
                "w1": w1,
                "b1": b1c,
                "w2": w2,
                "b2": b2c,
            }
        )
    res = run_bass_kernel_spmd(nc, in_maps, core_ids=list(range(N_CORES)))
    out = np.concatenate([r["outT"] for r in res.results], axis=1)
    return out[:, :Mtot]


def _seg_sum(x, seg, num):
    out = np.empty((num, x.shape[1]), np.float32)
    for j in range(x.shape[1]):
        out[:, j] = np.bincount(seg, weights=x[:, j], minlength=num)
    return out


def _seg_mean(x, seg, num):
    s = _seg_sum(x, seg, num)
    cnt = np.bincount(seg, minlength=num).astype(np.float32)
    return s / np.maximum(cnt, 1.0)[:, None]


def _sigmoid(x):
    return 1.0 / (1.0 + np.exp(-x))


def _set2set(x, seg, Bn, Wih, Whh, bih, bhh, steps=3):
    d = x.shape[1]
    q_star = np.zeros((Bn, 2 * d), np.float32)
    h = np.zeros((Bn, d), np.float32)
    c = np.zeros((Bn, d), np.float32)
    for _ in range(steps):
        gates = q_star @ Wih.T + bih + h @ Whh.T + bhh
        i, f, g, o = np.split(gates, 4, axis=1)
        c = _sigmoid(f) * c + _sigmoid(i) * np.tanh(g)
        h = _sigmoid(o) * np.tanh(c)
        q = h
        e = np.sum(x * q[seg], axis=1)
        emax = np.full((Bn,), -np.inf, np.float32)
        np.maximum.at(emax, seg, e)
        emax = np.where(np.isfinite(emax), emax, 0.0)
        ee = np.exp(e - emax[seg])
        denom = np.bincount(seg, weights=ee, minlength=Bn).astype(np.float32)
        a = ee / denom[seg]
        r = _seg_sum(a[:, None] * x, seg, Bn)
        q_star = np.concatenate([q, r], axis=1)
    return q_star


def kernel(
    node_features,
    edge_index,
    edge_features,
    global_features,
    batch,
    eW1,
    eb1,
    eW2,
    eb2,
    nW1,
    nb1,
    nW2,
    nb2,
    gW1,
    gb1,
    gW2,
    gb2,
    sn_Wih,
    sn_Whh,
    sn_bih,
    sn_bhh,
    se_Wih,
    se_Whh,
    se_bih,
    se_bhh,
    dW1,
    db1,
    dW2,
    db2,
    oW,
    ob,
):
    x = np.asarray(node_features, np.float32)
    ea = np.asarray(edge_features, np.float32)
    u = np.asarray(global_features, np.float32)
    ei = np.asarray(edge_index)
    batch = np.asarray(batch)
    N, E, Bn = x.shape[0], ea.shape[0], u.shape[0]

    src = np.concatenate([ei[0], ei[1]])
    dst = np.concatenate([ei[1], ei[0]])
    ebatch = batch[src]

    for i in range(N_BLOCKS):
        ea2 = np.concatenate([ea, ea], axis=0)
        e_in = np.concatenate([x[src], x[dst], ea2, u[ebatch]], axis=1)
        e_newT = _run_mlp(
            np.ascontiguousarray(e_in.T), eW1[i], eb1[i], eW2[i], eb2[i]
        )
        e_new = e_newT.T
        e_mean = _seg_mean(e_new, dst, N)
        n_in = np.concatenate([x, e_mean, u[batch]], axis=1)
        x_newT = _run_mlp(
            np.ascontiguousarray(n_in.T), nW1[i], nb1[i], nW2[i], nb2[i]
        )
        x_new = x_newT.T
        g_in = np.concatenate(
            [_seg_mean(e_new, ebatch, Bn), _seg_mean(x_new, batch, Bn), u], axis=1
        )
        u_new = (g_in @ gW1[i].T + gb1[i]) @ gW2[i].T + gb2[i]
        e_out = (e_new[:E] + e_new[E:]) * 0.5
        x = x_new + x
        ea = e_out + ea
        u = u_new + u

    xn = _set2set(x, batch, Bn, sn_Wih, sn_Whh, sn_bih, sn_bhh)
    xe = _set2set(ea, batch[ei[0]], Bn, se_Wih, se_Whh, se_bih, se_bhh)
    out = np.concatenate([xn, xe, u], axis=1)
    out = (out @ dW1.T + db1) @ dW2.T + db2
    out = out @ oW.T + ob
    return out.astype(np.float32)

